# revision 11
# baseline (speedup 1.0000x reference)
"""Trainium2 Bass kernel for the nn_Decoder LSTM-decoder problem.

Reference computation (per agent, 12 steps):
    gates = dec_in @ w_ih.T + h @ w_hh.T + (b_ih + b_hh)
    i, f, g, o = split(gates); c = sig(f)*c + sig(i)*tanh(g); h = sig(o)*tanh(c)
    rel = h @ w_hp.T + b_hp; dec_in = rel @ w_se.T + b_se
Output: rel per step, [12, N, 2].

Key algebraic fusion: dec_in_t is a linear function of h_t, so for steps >= 2
    gates_t = h_{t-1} @ W_eff.T + b_eff,  W_eff = w_hh + w_ih @ w_se @ w_hp
and step 1 uses w_hh plus U = w_ih @ w_se applied to last_pos_rel.
last_pos is dead (never affects the output).

Distribution: pure data parallel over the agent axis, 8192 agents per core
on 8 NeuronCores; weights replicated.

On-chip layout: [feature partitions, agent free]. Agents are processed in
1024-agent pairs (one [128, 1024] PSUM tile per gate) so each ACT
instruction covers 1024 elements per lane with the per-gate per-partition
bias fused. PE does float32r matmuls; DVE+GPSIMD split the cell-update
elementwise work. PSUM: gate tiles rotate through 3 slots (6 banks) and the
tiny rel matmul output has its own slot, so gate allocation never waits on
a prior pair's chain tail. The per-step rel output is re-blocked via
SBUF->SBUF DMA and pair-interleaved on DVE so the final DRAM write has
512-byte contiguous runs spread across all 16 DMA ports.
"""

import sys

if "/opt/trn_rl_repo" not in sys.path:
    sys.path.insert(0, "/opt/trn_rl_repo")

import numpy as np

T = 12          # steps
H = 128         # hidden dim
NCORES = 8
NPC = 8192      # agents per core
CH = 512        # agents per chunk (one PSUM bank at fp32)
PAIR = 2 * CH   # agents per gate-tile

_CACHE = {}


def _build_program(npc):
    import concourse.bass as bass
    import concourse.tile as tile
    from concourse import bacc, mybir

    dt = mybir.dt
    f32 = dt.float32
    f32r = dt.float32r
    Act = mybir.ActivationFunctionType

    npair = npc // PAIR
    assert npc % PAIR == 0
    nblk = npc // 64   # output partition blocks (64 agents each)

    nc = bacc.Bacc(
        "TRN2",
        target_bir_lowering=False,
        debug=False,
        num_devices=NCORES,
    )

    def din(name, shape, dt_=None):
        return nc.dram_tensor(
            name, list(shape), dt_ or f32, kind="ExternalInput"
        ).ap()

    h0_d = din("h0", [npc, H])
    c0_d = din("c0", [npc, H])
    lpr_d = din("lpr", [npc, 2])
    # lhsT layouts, K on partitions. Gate order [i, f, o, g].
    wg_d = din("wg", [H, 4 * H], f32r)    # W_eff.T columns gate-ordered
    whh_d = din("whh", [H, 4 * H], f32r)  # w_hh.T (step 1)
    u_d = din("u", [2, 4 * H], f32r)      # (w_ih @ w_se).T (step 1)
    bias_d = din("bias", [H, 8])          # ACT bias: [b_eff | b1] x [i,f,o,g]
    whp_d = din("whp", [H, 2], f32r)      # w_hp.T
    bhp_d = din("bhp", [2, 1])
    ident_d = din("ident", [H, H])
    out_d = nc.dram_tensor("out", [T, npc, 2], f32, kind="ExternalOutput").ap()

    with tile.TileContext(nc) as tc:
        with (
            tc.tile_pool(name="wpool", bufs=1) as wp,
            tc.tile_pool(name="state", bufs=1) as state,
            tc.tile_pool(name="stage", bufs=4) as stage,
            tc.tile_pool(name="sig", bufs=3) as sigp,
            tc.tile_pool(name="tmp", bufs=3) as tmpp,
            tc.tile_pool(name="outp", bufs=2) as outp,
            tc.tile_pool(name="ps", bufs=3, space="PSUM") as psp,
            tc.tile_pool(name="psr", bufs=1, space="PSUM") as psr,
        ):
            def wtile(ap, shape, tag, dt_=None):
                t_ = wp.tile(list(shape), dt_ or f32, tag=tag)
                nc.sync.dma_start(t_[:], ap)
                return t_

            wg = wtile(wg_d, [H, 4 * H], "wg", f32r)
            whh = wtile(whh_d, [H, 4 * H], "whh", f32r)
            u = wtile(u_d, [2, 4 * H], "u", f32r)
            bias = wtile(bias_d, [H, 8], "bias")
            whp = wtile(whp_d, [H, 2], "whp", f32r)
            bhp = wtile(bhp_d, [2, 1], "bhp")
            ident = wtile(ident_d, [H, H], "ident")

            h_sb = state.tile([H, npc], f32r, tag="h")
            c_sb = state.tile([H, npc], f32, tag="c")

            def step_pair(t, p, xblk, yblk, lpr_t):
                """Emit one (step, agent-pair) unit of the recurrence."""
                first = t == 0
                W = whh if first else wg
                bcol = 4 if first else 0
                cols = slice(p * PAIR, (p + 1) * PAIR)
                h_pr = h_sb[:, cols]
                c_pr = c_sb[:, cols]
                gt = [psp.tile([128, 1024], f32, tag="ps", name=f"gt{g}")
                      for g in range(4)]
                for g in range(4):
                    wsl = slice(g * H, (g + 1) * H)
                    for half in range(2):
                        hs = slice((p * 2 + half) * CH,
                                   (p * 2 + half + 1) * CH)
                        osl = slice(half * CH, (half + 1) * CH)
                        if first:
                            nc.tensor.matmul(
                                gt[g][:, osl], u[:, wsl],
                                lpr_t[:, osl],
                                start=True, stop=False)
                        nc.tensor.matmul(
                            gt[g][:, osl], W[:, wsl], h_sb[:, hs],
                            start=not first, stop=True)

                # activations, per-gate bias fused
                si = sigp.tile([128, PAIR], f32, tag="si")
                sf = sigp.tile([128, PAIR], f32, tag="sf")
                so = sigp.tile([128, PAIR], f32, tag="so")
                tg = sigp.tile([128, PAIR], f32, tag="tg")
                nc.scalar.activation(si[:], gt[0][:], Act.Sigmoid,
                                     bias=bias[:, bcol:bcol + 1])
                nc.scalar.activation(sf[:], gt[1][:], Act.Sigmoid,
                                     bias=bias[:, bcol + 1:bcol + 2])
                nc.scalar.activation(so[:], gt[2][:], Act.Sigmoid,
                                     bias=bias[:, bcol + 2:bcol + 3])
                nc.scalar.activation(tg[:], gt[3][:], Act.Tanh,
                                     bias=bias[:, bcol + 3:bcol + 4])

                # cell update: c = sf*c + si*tg ; h = so*tanh(c)
                m1 = tmpp.tile([128, PAIR], f32, tag="m1")
                nc.vector.tensor_mul(m1[:], sf[:], c_pr)
                m2 = tmpp.tile([128, PAIR], f32, tag="m2")
                nc.vector.tensor_mul(m2[:], si[:], tg[:])
                nc.gpsimd.tensor_add(c_pr, m1[:], m2[:])
                tcl = sigp.tile([128, PAIR], f32, tag="tc")
                nc.scalar.activation(tcl[:], c_pr, Act.Tanh)
                nc.vector.tensor_mul(
                    h_pr[:, 0:256], so[:, 0:256], tcl[:, 0:256])
                nc.gpsimd.tensor_mul(
                    h_pr[:, 256:PAIR], so[:, 256:PAIR], tcl[:, 256:PAIR])

            def rel_pair(p, xblk, yblk):
                # rel = w_hp @ h + b_hp  -> [2, PAIR] psum
                rp = psr.tile([2, 1024], f32, tag="rel")
                for half in range(2):
                    hs = slice((p * 2 + half) * CH,
                               (p * 2 + half + 1) * CH)
                    osl = slice(half * CH, (half + 1) * CH)
                    nc.tensor.matmul(
                        rp[0:2, osl], whp[:], h_sb[:, hs],
                        start=True, stop=True)
                ex = tmpp.tile([2, PAIR], f32, tag="ex")
                nc.vector.tensor_scalar_add(ex[:], rp[0:2, :], bhp[:, 0:1])
                # re-block: agent a -> partition a//64; pair p covers
                # partitions [16p, 16p+16)
                prt = slice(16 * p, 16 * (p + 1))
                nc.sync.dma_start(xblk[prt, :], ex[0:1, :])
                nc.sync.dma_start(yblk[prt, :], ex[1:2, :])

            def flush_step(t, xblk, yblk):
                # interleave x/y pairs per partition and write out:
                # out[t, 64q + a, k] <- relpk[q, 2a + k]
                relpk = outp.tile([nblk, 128], f32, tag="relpk")
                rv = relpk[:].rearrange("q (a k) -> q a k", k=2)
                nc.vector.tensor_copy(rv[:, :, 0], xblk[:])
                nc.vector.tensor_copy(rv[:, :, 1], yblk[:])
                nc.sync.dma_start(
                    out_d[t].rearrange("(q a) k -> q (a k)", a=64), relpk[:])

            # ---- prologue + step 0, software-pipelined per pair ----
            def prologue_pair(p):
                cols = slice(p * PAIR, (p + 1) * PAIR)
                pt_h = psp.tile([128, 1024], f32, tag="ps")
                pt_c = psp.tile([128, 1024], f32, tag="ps")
                pt_l = psp.tile([128, 1024], f32, tag="ps")
                for j in range(8):
                    rows = slice(p * PAIR + j * 128, p * PAIR + (j + 1) * 128)
                    st = stage.tile([128, H], f32, tag="st_h")
                    nc.sync.dma_start(st[:], h0_d[rows, :])
                    nc.tensor.transpose(
                        pt_h[:, j * 128:(j + 1) * 128], st[:], ident[:])
                    st = stage.tile([128, H], f32, tag="st_c")
                    nc.sync.dma_start(st[:], c0_d[rows, :])
                    nc.tensor.transpose(
                        pt_c[:, j * 128:(j + 1) * 128], st[:], ident[:])
                    st = stage.tile([128, 2], f32, tag="st_l")
                    nc.sync.dma_start(st[:], lpr_d[rows, :])
                    nc.tensor.transpose(
                        pt_l[0:2, j * 128:(j + 1) * 128], st[:], ident[:])
                nc.vector.tensor_copy(h_sb[:, cols], pt_h[:])
                nc.vector.tensor_copy(c_sb[:, cols], pt_c[:])
                lpr_t = tmpp.tile([2, PAIR], f32r, tag="lprp", bufs=2)
                nc.vector.tensor_copy(lpr_t[:], pt_l[0:2, :])
                return lpr_t

            xblk = outp.tile([nblk, 64], f32, tag="xblk")
            yblk = outp.tile([nblk, 64], f32, tag="yblk")
            lpr_tiles = {0: prologue_pair(0)}
            for p in range(npair):
                if p + 1 < npair:
                    lpr_tiles[p + 1] = prologue_pair(p + 1)
                step_pair(0, p, xblk, yblk, lpr_tiles.pop(p))
                if p > 0:
                    rel_pair(p - 1, xblk, yblk)
            rel_pair(npair - 1, xblk, yblk)
            flush_step(0, xblk, yblk)

            # ---- steps 1..T-1, rel stage one pair behind ----
            for t in range(1, T):
                xblk = outp.tile([nblk, 64], f32, tag="xblk")
                yblk = outp.tile([nblk, 64], f32, tag="yblk")
                for p in range(npair):
                    step_pair(t, p, xblk, yblk, None)
                    if p > 0:
                        rel_pair(p - 1, xblk, yblk)
                rel_pair(npair - 1, xblk, yblk)
                flush_step(t, xblk, yblk)

    nc.compile()
    return nc


def _fold_weights(w_ih, w_hh, b_ih, b_hh, w_se, b_se, w_hp, b_hp):
    """Host-side constant folding. Gate order [i, f, o, g] (torch order in
    the 4H rows is i, f, g, o)."""
    perm = np.concatenate([
        np.arange(0, H), np.arange(H, 2 * H),
        np.arange(3 * H, 4 * H), np.arange(2 * H, 3 * H),
    ])
    W_eff = w_hh + w_ih @ w_se @ w_hp                      # [4H, H]
    b_eff = (b_hp @ w_se.T + b_se) @ w_ih.T + b_ih + b_hh  # [4H]
    U = w_ih @ w_se                                        # [4H, 2]
    b1 = b_se @ w_ih.T + b_ih + b_hh                       # [4H]

    Wp, bp = W_eff[perm], b_eff[perm]
    Whhp, Up, b1p = w_hh[perm], U[perm], b1[perm]
    f = np.float32
    bias = np.stack([bp[0:H], bp[H:2*H], bp[2*H:3*H], bp[3*H:4*H],
                     b1p[0:H], b1p[H:2*H], b1p[2*H:3*H], b1p[3*H:4*H]],
                    axis=1)  # [H, 8]
    return {
        "wg": np.ascontiguousarray(Wp.T, f),
        "whh": np.ascontiguousarray(Whhp.T, f),
        "u": np.ascontiguousarray(Up.T, f),
        "bias": np.ascontiguousarray(bias, f),
        "whp": np.ascontiguousarray(w_hp.T, f),
        "bhp": np.ascontiguousarray(np.asarray(b_hp)[:, None], f),
        "ident": np.eye(H, dtype=f),
    }


def kernel(last_pos, last_pos_rel, h0, c0,
           w_ih, w_hh, b_ih, b_hh, w_se, b_se, w_hp, b_hp):
    last_pos_rel = np.ascontiguousarray(np.asarray(last_pos_rel), np.float32)
    h0 = np.ascontiguousarray(np.asarray(h0), np.float32)
    c0 = np.ascontiguousarray(np.asarray(c0), np.float32)
    consts = _fold_weights(
        np.asarray(w_ih, np.float32), np.asarray(w_hh, np.float32),
        np.asarray(b_ih, np.float32), np.asarray(b_hh, np.float32),
        np.asarray(w_se, np.float32), np.asarray(b_se, np.float32),
        np.asarray(w_hp, np.float32), np.asarray(b_hp, np.float32),
    )

    npeds = h0.shape[0]
    npc = npeds // NCORES
    if "nc" not in _CACHE or _CACHE.get("npc") != npc:
        _CACHE["nc"] = _build_program(npc)
        _CACHE["npc"] = npc
    nc = _CACHE["nc"]

    in_maps = []
    for ci in range(NCORES):
        rows = slice(ci * npc, (ci + 1) * npc)
        m = {"h0": h0[rows], "c0": c0[rows], "lpr": last_pos_rel[rows]}
        m.update(consts)
        in_maps.append(m)

    from concourse.bass_utils import run_bass_kernel_spmd
    import os

    res = run_bass_kernel_spmd(
        nc, in_maps, list(range(NCORES)),
        tmpdir=os.environ.get("KERNEL_TRACE_DIR"),
    )
    _CACHE["exec_time_ns"] = res.exec_time_ns
    _CACHE["results"] = res
    outs = [np.asarray(res.results[i]["out"]) for i in range(NCORES)]
    return np.concatenate(outs, axis=1)


# revision 13
# speedup vs baseline: 1.2084x; 1.2084x over previous
"""Trainium2 Bass kernel for the nn_Decoder LSTM-decoder problem.

Reference computation (per agent, 12 steps):
    gates = dec_in @ w_ih.T + h @ w_hh.T + (b_ih + b_hh)
    i, f, g, o = split(gates); c = sig(f)*c + sig(i)*tanh(g); h = sig(o)*tanh(c)
    rel = h @ w_hp.T + b_hp; dec_in = rel @ w_se.T + b_se
Output: rel per step, [12, N, 2].

Key algebraic fusion: dec_in_t is a linear function of h_t, so for steps >= 2
    gates_t = h_{t-1} @ W_eff.T + b_eff,  W_eff = w_hh + w_ih @ w_se @ w_hp
and step 1 uses w_hh plus U = w_ih @ w_se applied to last_pos_rel.
last_pos is dead (never affects the output).

Distribution: pure data parallel over the agent axis, 8192 agents per core
on 8 NeuronCores; weights replicated.

On-chip layout: [feature partitions, agent free]. Agents are processed in
1024-agent pairs (one [128, 1024] PSUM tile per gate) so each ACT
instruction covers 1024 elements per lane with the per-gate per-partition
bias fused. PE does float32r matmuls; DVE+GPSIMD split the cell-update
elementwise work. PSUM: gate tiles rotate through 3 slots (6 banks) and the
tiny rel matmul output has its own slot, so gate allocation never waits on
a prior pair's chain tail. The per-step rel output is re-blocked via
SBUF->SBUF DMA and pair-interleaved on DVE so the final DRAM write has
512-byte contiguous runs spread across all 16 DMA ports.
"""

import sys

if "/opt/trn_rl_repo" not in sys.path:
    sys.path.insert(0, "/opt/trn_rl_repo")

import numpy as np

T = 12          # steps
H = 128         # hidden dim
NCORES = 8
NPC = 8192      # agents per core
CH = 512        # agents per chunk (one PSUM bank at fp32)
PAIR = 2 * CH   # agents per gate-tile

_CACHE = {}


def _build_program(npc):
    import concourse.bass as bass
    import concourse.tile as tile
    from concourse import bacc, mybir

    dt = mybir.dt
    f32 = dt.float32
    f32r = dt.float32r
    Act = mybir.ActivationFunctionType

    npair = npc // PAIR
    assert npc % PAIR == 0
    nblk = npc // 64   # output partition blocks (64 agents each)

    nc = bacc.Bacc(
        "TRN2",
        target_bir_lowering=False,
        debug=False,
        num_devices=NCORES,
    )

    def din(name, shape, dt_=None):
        return nc.dram_tensor(
            name, list(shape), dt_ or f32, kind="ExternalInput"
        ).ap()

    h0_d = din("h0", [npc, H])
    c0_d = din("c0", [npc, H])
    lpr_d = din("lpr", [npc, 2])
    # lhsT layouts, K on partitions. Gate order [i, f, o, g].
    wg_d = din("wg", [H, 4 * H], f32r)    # W_eff.T columns gate-ordered
    whh_d = din("whh", [H, 4 * H], f32r)  # w_hh.T (step 1)
    u_d = din("u", [2, 4 * H], f32r)      # (w_ih @ w_se).T (step 1)
    bias_d = din("bias", [H, 8])          # ACT bias: [b_eff | b1] x [i,f,o,g]
    whp_d = din("whp", [H, 2], f32r)      # w_hp.T
    bhp_d = din("bhp", [2, 1])
    ident_d = din("ident", [H, H])
    out_d = nc.dram_tensor("out", [T, npc, 2], f32, kind="ExternalOutput").ap()

    with tile.TileContext(nc) as tc:
        with (
            tc.tile_pool(name="wpool", bufs=1) as wp,
            tc.tile_pool(name="state", bufs=1) as state,
            tc.tile_pool(name="stage", bufs=4) as stage,
            tc.tile_pool(name="sig", bufs=3) as sigp,
            tc.tile_pool(name="tmp", bufs=3) as tmpp,
            tc.tile_pool(name="outp", bufs=2) as outp,
            tc.tile_pool(name="ps", bufs=3, space="PSUM") as psp,
            tc.tile_pool(name="psr", bufs=1, space="PSUM") as psr,
        ):
            def wtile(ap, shape, tag, dt_=None):
                t_ = wp.tile(list(shape), dt_ or f32, tag=tag)
                nc.sync.dma_start(t_[:], ap)
                return t_

            wg = wtile(wg_d, [H, 4 * H], "wg", f32r)
            whh = wtile(whh_d, [H, 4 * H], "whh", f32r)
            u = wtile(u_d, [2, 4 * H], "u", f32r)
            bias = wtile(bias_d, [H, 8], "bias")
            whp = wtile(whp_d, [H, 2], "whp", f32r)
            bhp = wtile(bhp_d, [2, 1], "bhp")
            ident = wtile(ident_d, [H, H], "ident")

            h_sb = state.tile([H, npc], f32r, tag="h")
            c_sb = state.tile([H, npc], f32, tag="c")

            def step_pair(t, p, xblk, yblk, lpr_t):
                """Emit one (step, agent-pair) unit of the recurrence."""
                first = t == 0
                W = whh if first else wg
                bcol = 4 if first else 0
                cols = slice(p * PAIR, (p + 1) * PAIR)
                h_pr = h_sb[:, cols]
                c_pr = c_sb[:, cols]
                gt = [psp.tile([128, 1024], f32, tag="ps", name=f"gt{g}")
                      for g in range(4)]
                last_mm = [None]
                for g in range(4):
                    wsl = slice(g * H, (g + 1) * H)
                    for half in range(2):
                        hs = slice((p * 2 + half) * CH,
                                   (p * 2 + half + 1) * CH)
                        osl = slice(half * CH, (half + 1) * CH)
                        if first:
                            nc.tensor.matmul(
                                gt[g][:, osl], u[:, wsl],
                                lpr_t[:, osl],
                                start=True, stop=False)
                        last_mm[0] = nc.tensor.matmul(
                            gt[g][:, osl], W[:, wsl], h_sb[:, hs],
                            start=not first, stop=True)

                # activations, per-gate bias fused
                si = sigp.tile([128, PAIR], f32, tag="si")
                sf = sigp.tile([128, PAIR], f32, tag="sf")
                so = sigp.tile([128, PAIR], f32, tag="so")
                tg = sigp.tile([128, PAIR], f32, tag="tg")
                nc.scalar.activation(si[:], gt[0][:], Act.Sigmoid,
                                     bias=bias[:, bcol:bcol + 1])
                nc.scalar.activation(sf[:], gt[1][:], Act.Sigmoid,
                                     bias=bias[:, bcol + 1:bcol + 2])
                nc.scalar.activation(so[:], gt[2][:], Act.Sigmoid,
                                     bias=bias[:, bcol + 2:bcol + 3])
                nc.scalar.activation(tg[:], gt[3][:], Act.Tanh,
                                     bias=bias[:, bcol + 3:bcol + 4])

                # cell update: c = sf*c + si*tg ; h = so*tanh(c)
                m1 = tmpp.tile([128, PAIR], f32, tag="m1")
                nc.vector.tensor_mul(m1[:], sf[:], c_pr)
                m2 = tmpp.tile([128, PAIR], f32, tag="m2")
                nc.vector.tensor_mul(m2[:], si[:], tg[:])
                nc.gpsimd.tensor_add(c_pr, m1[:], m2[:])
                tcl = sigp.tile([128, PAIR], f32, tag="tc")
                nc.scalar.activation(tcl[:], c_pr, Act.Tanh)
                nc.vector.tensor_mul(
                    h_pr[:, 0:CH], so[:, 0:CH], tcl[:, 0:CH])
                nc.gpsimd.tensor_mul(
                    h_pr[:, CH:PAIR], so[:, CH:PAIR], tcl[:, CH:PAIR])
                return last_mm[0]

            from concourse.tile_rust import add_dep_helper

            def rel_pair(p, xblk, yblk, after=None):
                # rel = w_hp @ h + b_hp  -> [2, PAIR] psum
                rp = psr.tile([2, 1024], f32, tag="rel")
                for half in range(2):
                    hs = slice((p * 2 + half) * CH,
                               (p * 2 + half + 1) * CH)
                    osl = slice(half * CH, (half + 1) * CH)
                    mm = nc.tensor.matmul(
                        rp[0:2, osl], whp[:], h_sb[:, hs],
                        start=True, stop=True)
                    if after is not None:
                        add_dep_helper(mm.ins, after.ins, sync=False,
                                       reason="rel after next pair gates")
                ex = tmpp.tile([2, PAIR], f32, tag="ex")
                nc.vector.tensor_scalar_add(ex[:], rp[0:2, :], bhp[:, 0:1])
                # re-block: agent a -> partition a//64; pair p covers
                # partitions [16p, 16p+16)
                prt = slice(16 * p, 16 * (p + 1))
                nc.sync.dma_start(xblk[prt, :], ex[0:1, :])
                nc.sync.dma_start(yblk[prt, :], ex[1:2, :])

            def flush_step(t, xblk, yblk):
                # interleave x/y pairs per partition and write out:
                # out[t, 64q + a, k] <- relpk[q, 2a + k]
                relpk = outp.tile([nblk, 128], f32, tag="relpk")
                rv = relpk[:].rearrange("q (a k) -> q a k", k=2)
                nc.vector.tensor_copy(rv[:, :, 0], xblk[:])
                nc.vector.tensor_copy(rv[:, :, 1], yblk[:])
                nc.sync.dma_start(
                    out_d[t].rearrange("(q a) k -> q (a k)", a=64), relpk[:])

            # ---- prologue + step 0, software-pipelined per pair ----
            def prologue_pair(p):
                cols = slice(p * PAIR, (p + 1) * PAIR)
                pt_h = psp.tile([128, 1024], f32, tag="ps")
                pt_c = psp.tile([128, 1024], f32, tag="ps")
                pt_l = psp.tile([128, 1024], f32, tag="ps")
                for j in range(8):
                    rows = slice(p * PAIR + j * 128, p * PAIR + (j + 1) * 128)
                    st = stage.tile([128, H], f32, tag="st_h")
                    nc.sync.dma_start(st[:], h0_d[rows, :])
                    nc.tensor.transpose(
                        pt_h[:, j * 128:(j + 1) * 128], st[:], ident[:])
                    st = stage.tile([128, H], f32, tag="st_c")
                    nc.sync.dma_start(st[:], c0_d[rows, :])
                    nc.tensor.transpose(
                        pt_c[:, j * 128:(j + 1) * 128], st[:], ident[:])
                    st = stage.tile([128, 2], f32, tag="st_l")
                    nc.sync.dma_start(st[:], lpr_d[rows, :])
                    nc.tensor.transpose(
                        pt_l[0:2, j * 128:(j + 1) * 128], st[:], ident[:])
                nc.vector.tensor_copy(h_sb[:, cols], pt_h[:])
                nc.vector.tensor_copy(c_sb[:, cols], pt_c[:])
                lpr_t = tmpp.tile([2, PAIR], f32r, tag="lprp", bufs=2)
                nc.vector.tensor_copy(lpr_t[:], pt_l[0:2, :])
                return lpr_t

            xblk = outp.tile([nblk, 64], f32, tag="xblk")
            yblk = outp.tile([nblk, 64], f32, tag="yblk")
            lpr_tiles = {0: prologue_pair(0)}
            for p in range(npair):
                if p + 1 < npair:
                    lpr_tiles[p + 1] = prologue_pair(p + 1)
                mm = step_pair(0, p, xblk, yblk, lpr_tiles.pop(p))
                if p > 0:
                    rel_pair(p - 1, xblk, yblk, after=mm)
            rel_pair(npair - 1, xblk, yblk)
            flush_step(0, xblk, yblk)

            # ---- steps 1..T-1, rel stage one pair behind ----
            for t in range(1, T):
                xblk = outp.tile([nblk, 64], f32, tag="xblk")
                yblk = outp.tile([nblk, 64], f32, tag="yblk")
                for p in range(npair):
                    mm = step_pair(t, p, xblk, yblk, None)
                    if p > 0:
                        rel_pair(p - 1, xblk, yblk, after=mm)
                rel_pair(npair - 1, xblk, yblk)
                flush_step(t, xblk, yblk)

    nc.compile()
    return nc


def _fold_weights(w_ih, w_hh, b_ih, b_hh, w_se, b_se, w_hp, b_hp):
    """Host-side constant folding. Gate order [i, f, o, g] (torch order in
    the 4H rows is i, f, g, o)."""
    perm = np.concatenate([
        np.arange(0, H), np.arange(H, 2 * H),
        np.arange(3 * H, 4 * H), np.arange(2 * H, 3 * H),
    ])
    W_eff = w_hh + w_ih @ w_se @ w_hp                      # [4H, H]
    b_eff = (b_hp @ w_se.T + b_se) @ w_ih.T + b_ih + b_hh  # [4H]
    U = w_ih @ w_se                                        # [4H, 2]
    b1 = b_se @ w_ih.T + b_ih + b_hh                       # [4H]

    Wp, bp = W_eff[perm], b_eff[perm]
    Whhp, Up, b1p = w_hh[perm], U[perm], b1[perm]
    f = np.float32
    bias = np.stack([bp[0:H], bp[H:2*H], bp[2*H:3*H], bp[3*H:4*H],
                     b1p[0:H], b1p[H:2*H], b1p[2*H:3*H], b1p[3*H:4*H]],
                    axis=1)  # [H, 8]
    return {
        "wg": np.ascontiguousarray(Wp.T, f),
        "whh": np.ascontiguousarray(Whhp.T, f),
        "u": np.ascontiguousarray(Up.T, f),
        "bias": np.ascontiguousarray(bias, f),
        "whp": np.ascontiguousarray(w_hp.T, f),
        "bhp": np.ascontiguousarray(np.asarray(b_hp)[:, None], f),
        "ident": np.eye(H, dtype=f),
    }


def kernel(last_pos, last_pos_rel, h0, c0,
           w_ih, w_hh, b_ih, b_hh, w_se, b_se, w_hp, b_hp):
    last_pos_rel = np.ascontiguousarray(np.asarray(last_pos_rel), np.float32)
    h0 = np.ascontiguousarray(np.asarray(h0), np.float32)
    c0 = np.ascontiguousarray(np.asarray(c0), np.float32)
    consts = _fold_weights(
        np.asarray(w_ih, np.float32), np.asarray(w_hh, np.float32),
        np.asarray(b_ih, np.float32), np.asarray(b_hh, np.float32),
        np.asarray(w_se, np.float32), np.asarray(b_se, np.float32),
        np.asarray(w_hp, np.float32), np.asarray(b_hp, np.float32),
    )

    npeds = h0.shape[0]
    npc = npeds // NCORES
    if "nc" not in _CACHE or _CACHE.get("npc") != npc:
        _CACHE["nc"] = _build_program(npc)
        _CACHE["npc"] = npc
    nc = _CACHE["nc"]

    in_maps = []
    for ci in range(NCORES):
        rows = slice(ci * npc, (ci + 1) * npc)
        m = {"h0": h0[rows], "c0": c0[rows], "lpr": last_pos_rel[rows]}
        m.update(consts)
        in_maps.append(m)

    from concourse.bass_utils import run_bass_kernel_spmd
    import os

    res = run_bass_kernel_spmd(
        nc, in_maps, list(range(NCORES)),
        tmpdir=os.environ.get("KERNEL_TRACE_DIR"),
    )
    _CACHE["exec_time_ns"] = res.exec_time_ns
    _CACHE["results"] = res
    outs = [np.asarray(res.results[i]["out"]) for i in range(NCORES)]
    return np.concatenate(outs, axis=1)


# revision 16
# speedup vs baseline: 1.2139x; 1.0045x over previous
"""Trainium2 Bass kernel for the nn_Decoder LSTM-decoder problem.

Reference computation (per agent, 12 steps):
    gates = dec_in @ w_ih.T + h @ w_hh.T + (b_ih + b_hh)
    i, f, g, o = split(gates); c = sig(f)*c + sig(i)*tanh(g); h = sig(o)*tanh(c)
    rel = h @ w_hp.T + b_hp; dec_in = rel @ w_se.T + b_se
Output: rel per step, [12, N, 2].

Key algebraic fusion: dec_in_t is a linear function of h_t, so for steps >= 2
    gates_t = h_{t-1} @ W_eff.T + b_eff,  W_eff = w_hh + w_ih @ w_se @ w_hp
and step 1 uses w_hh plus U = w_ih @ w_se applied to last_pos_rel.
last_pos is dead (never affects the output).

Distribution: pure data parallel over the agent axis, 8192 agents per core
on 8 NeuronCores; weights replicated.

On-chip layout: [feature partitions, agent free]. Agents are processed in
1024-agent pairs (one [128, 1024] PSUM tile per gate) so each ACT
instruction covers 1024 elements per lane with the per-gate per-partition
bias fused. PE does float32r matmuls; DVE+GPSIMD split the cell-update
elementwise work. PSUM: gate tiles rotate through 3 slots (6 banks) and the
tiny rel matmul output has its own slot, so gate allocation never waits on
a prior pair's chain tail. The per-step rel output is re-blocked via
SBUF->SBUF DMA and pair-interleaved on DVE so the final DRAM write has
512-byte contiguous runs spread across all 16 DMA ports.
"""

import sys

if "/opt/trn_rl_repo" not in sys.path:
    sys.path.insert(0, "/opt/trn_rl_repo")

import numpy as np

T = 12          # steps
H = 128         # hidden dim
NCORES = 8
NPC = 8192      # agents per core
CH = 512        # agents per chunk (one PSUM bank at fp32)
PAIR = 2 * CH   # agents per gate-tile

_CACHE = {}


def _build_program(npc):
    import concourse.bass as bass
    import concourse.tile as tile
    from concourse import bacc, mybir

    dt = mybir.dt
    f32 = dt.float32
    f32r = dt.float32r
    Act = mybir.ActivationFunctionType

    npair = npc // PAIR
    assert npc % PAIR == 0 and npc >= 2 * PAIR
    nblk = npc // 64   # output partition blocks (64 agents each)

    nc = bacc.Bacc(
        "TRN2",
        target_bir_lowering=False,
        debug=False,
        num_devices=NCORES,
    )

    def din(name, shape, dt_=None):
        return nc.dram_tensor(
            name, list(shape), dt_ or f32, kind="ExternalInput"
        ).ap()

    h0_d = din("h0", [npc, H])
    c0_d = din("c0", [npc, H])
    lpr_d = din("lpr", [npc, 2])
    # lhsT layouts, K on partitions. Gate order [i, f, o, g].
    wg_d = din("wg", [H, 4 * H], f32r)    # W_eff.T columns gate-ordered
    whh_d = din("whh", [H, 4 * H], f32r)  # w_hh.T (step 1)
    u_d = din("u", [2, 4 * H], f32r)      # (w_ih @ w_se).T (step 1)
    bias_d = din("bias", [H, 8])          # ACT bias: [b_eff | b1] x [i,f,o,g]
    whp_d = din("whp", [H, 2], f32r)      # w_hp.T
    bhp_d = din("bhp", [128, 1])
    ident_d = din("ident", [H, H])
    out_d = nc.dram_tensor("out", [T, npc, 2], f32, kind="ExternalOutput").ap()

    with tile.TileContext(nc) as tc:
        with (
            tc.tile_pool(name="wpool", bufs=1) as wp,
            tc.tile_pool(name="state", bufs=1) as state,
            tc.tile_pool(name="stage", bufs=4) as stage,
            tc.tile_pool(name="sig", bufs=3) as sigp,
            tc.tile_pool(name="tmp", bufs=3) as tmpp,
            tc.tile_pool(name="outp", bufs=2) as outp,
            tc.tile_pool(name="ps", bufs=3, space="PSUM") as psp,
            tc.tile_pool(name="psr", bufs=1, space="PSUM") as psr,
        ):
            def wtile(ap, shape, tag, dt_=None):
                t_ = wp.tile(list(shape), dt_ or f32, tag=tag)
                nc.sync.dma_start(t_[:], ap)
                return t_

            wg = wtile(wg_d, [H, 4 * H], "wg", f32r)
            whh = wtile(whh_d, [H, 4 * H], "whh", f32r)
            u = wtile(u_d, [2, 4 * H], "u", f32r)
            bias = wtile(bias_d, [H, 8], "bias")
            whp = wtile(whp_d, [H, 2], "whp", f32r)
            bhp = wtile(bhp_d, [128, 1], "bhp")
            ident = wtile(ident_d, [H, H], "ident")

            h_sb = state.tile([H, npc], f32r, tag="h")
            c_sb = state.tile([H, npc], f32, tag="c")

            from concourse.tile_rust import add_dep_helper

            def front(t, p, lpr_t):
                """Gates + sigma_i/sigma_f/tanh_g + m1/m2 + sigma_o + c-add."""
                first = t == 0
                W = whh if first else wg
                bcol = 4 if first else 0
                cols = slice(p * PAIR, (p + 1) * PAIR)
                c_pr = c_sb[:, cols]
                gt = {}
                # allocation order matches ACT consumption order:
                # wg column groups are [i, f, o, g] -> alloc i(0), f(1), g(3), o(2)
                for g in (0, 1, 3, 2):
                    gt[g] = psp.tile([128, 1024], f32, tag="ps", name=f"gt{g}")
                    wsl = slice(g * H, (g + 1) * H)
                    for half in range(2):
                        hs = slice((p * 2 + half) * CH,
                                   (p * 2 + half + 1) * CH)
                        osl = slice(half * CH, (half + 1) * CH)
                        if first:
                            nc.tensor.matmul(
                                gt[g][:, osl], u[:, wsl], lpr_t[:, osl],
                                start=True, stop=False)
                        nc.tensor.matmul(
                            gt[g][:, osl], W[:, wsl], h_sb[:, hs],
                            start=not first, stop=True)

                si = sigp.tile([128, PAIR], f32, tag="si")
                sf = sigp.tile([128, PAIR], f32, tag="sf")
                tg = sigp.tile([128, PAIR], f32, tag="tg")
                nc.scalar.activation(si[:], gt[0][:], Act.Sigmoid,
                                     bias=bias[:, bcol:bcol + 1])
                nc.scalar.activation(sf[:], gt[1][:], Act.Sigmoid,
                                     bias=bias[:, bcol + 1:bcol + 2])
                nc.scalar.activation(tg[:], gt[3][:], Act.Tanh,
                                     bias=bias[:, bcol + 3:bcol + 4])
                m1 = tmpp.tile([128, PAIR], f32, tag="m1")
                nc.vector.tensor_mul(m1[:], sf[:], c_pr)
                m2 = tmpp.tile([128, PAIR], f32, tag="m2")
                nc.vector.tensor_mul(m2[:], si[:], tg[:])
                so = sigp.tile([128, PAIR], f32, tag="so")
                nc.scalar.activation(so[:], gt[2][:], Act.Sigmoid,
                                     bias=bias[:, bcol + 2:bcol + 3])
                nc.gpsimd.tensor_add(c_pr, m1[:], m2[:])
                return so

            def back(t, p, so):
                """tanh(c) + h update (deferred one unit)."""
                cols = slice(p * PAIR, (p + 1) * PAIR)
                h_pr = h_sb[:, cols]
                c_pr = c_sb[:, cols]
                tcl = sigp.tile([128, PAIR], f32, tag="tc")
                nc.scalar.activation(tcl[:], c_pr, Act.Tanh)
                nc.gpsimd.tensor_mul(h_pr, so[:], tcl[:])

            def rel_pair(t, p, blks):
                """rel = w_hp @ h + b_hp (deferred two units).
                Halves col-packed at psum partitions 0 and 32."""
                xblk, yblk = blks
                rp = psr.tile([2, 1024], f32, tag="rel")
                for half in range(2):
                    hs = slice((p * 2 + half) * CH,
                               (p * 2 + half + 1) * CH)
                    osl = slice(half * CH, (half + 1) * CH)
                    nc.tensor.matmul(
                        rp[0:2, osl], whp[:], h_sb[:, hs],
                        start=True, stop=True)
                ex = tmpp.tile([2, PAIR], f32, tag="ex")
                nc.vector.tensor_scalar_add(ex[:], rp[:], bhp[0:2, 0:1])
                prt = slice(16 * p, 16 * (p + 1))
                nc.sync.dma_start(xblk[prt, :], ex[0:1, :])
                nc.sync.dma_start(yblk[prt, :], ex[1:2, :])

            def flush_step(t, blks):
                xblk, yblk = blks
                relpk = outp.tile([nblk, 128], f32, tag="relpk")
                rv = relpk[:].rearrange("q (a k) -> q a k", k=2)
                nc.vector.tensor_copy(rv[:, :, 0], xblk[:])
                nc.vector.tensor_copy(rv[:, :, 1], yblk[:])
                nc.sync.dma_start(
                    out_d[t].rearrange("(q a) k -> q (a k)", a=64), relpk[:])

            def prologue_pair(p):
                cols = slice(p * PAIR, (p + 1) * PAIR)
                pt_h = psp.tile([128, 1024], f32, tag="ps")
                pt_c = psp.tile([128, 1024], f32, tag="ps")
                pt_l = psp.tile([128, 1024], f32, tag="ps")
                for j in range(8):
                    rows = slice(p * PAIR + j * 128, p * PAIR + (j + 1) * 128)
                    st = stage.tile([128, H], f32, tag="st_h")
                    nc.sync.dma_start(st[:], h0_d[rows, :])
                    nc.tensor.transpose(
                        pt_h[:, j * 128:(j + 1) * 128], st[:], ident[:])
                    st = stage.tile([128, H], f32, tag="st_c")
                    nc.sync.dma_start(st[:], c0_d[rows, :])
                    nc.tensor.transpose(
                        pt_c[:, j * 128:(j + 1) * 128], st[:], ident[:])
                    st = stage.tile([128, 2], f32, tag="st_l")
                    nc.sync.dma_start(st[:], lpr_d[rows, :])
                    nc.tensor.transpose(
                        pt_l[0:2, j * 128:(j + 1) * 128], st[:], ident[:])
                nc.vector.tensor_copy(h_sb[:, cols], pt_h[:])
                nc.vector.tensor_copy(c_sb[:, cols], pt_c[:])
                lpr_t = tmpp.tile([2, PAIR], f32r, tag="lprp", bufs=2)
                nc.vector.tensor_copy(lpr_t[:], pt_l[0:2, :])
                return lpr_t

            # ---- unit pipeline: FRONT(k) | BACK(k-1) | REL(k-2) ----
            units = [(t, p) for t in range(T) for p in range(npair)]
            blks = {}
            pend_back = []   # (t, p, so)
            pend_rel = []    # (t, p)
            done_pairs = {t: 0 for t in range(T)}
            lpr_next = prologue_pair(0)

            def emit_rel(t, p):
                rel_pair(t, p, blks[t])
                done_pairs[t] += 1
                if done_pairs[t] == npair:
                    flush_step(t, blks.pop(t))

            for k, (t, p) in enumerate(units):
                if t not in blks:
                    xb = outp.tile([nblk, 64], f32, tag="xblk",
                                   name=f"xb{t}")
                    yb = outp.tile([nblk, 64], f32, tag="yblk",
                                   name=f"yb{t}")
                    blks[t] = (xb, yb)
                lpr_t = None
                if t == 0:
                    lpr_t = lpr_next
                    if p + 1 < npair:
                        lpr_next = prologue_pair(p + 1)
                so = front(t, p, lpr_t)
                if pend_back:
                    back(*pend_back.pop(0))
                pend_back.append((t, p, so))
                if pend_rel:
                    emit_rel(*pend_rel.pop(0))
                pend_rel.append((t, p))
            while pend_back:
                back(*pend_back.pop(0))
            while pend_rel:
                emit_rel(*pend_rel.pop(0))

    nc.compile()
    return nc


def _fold_weights(w_ih, w_hh, b_ih, b_hh, w_se, b_se, w_hp, b_hp):
    """Host-side constant folding. Gate order [i, f, o, g] (torch order in
    the 4H rows is i, f, g, o)."""
    perm = np.concatenate([
        np.arange(0, H), np.arange(H, 2 * H),
        np.arange(3 * H, 4 * H), np.arange(2 * H, 3 * H),
    ])
    W_eff = w_hh + w_ih @ w_se @ w_hp                      # [4H, H]
    b_eff = (b_hp @ w_se.T + b_se) @ w_ih.T + b_ih + b_hh  # [4H]
    U = w_ih @ w_se                                        # [4H, 2]
    b1 = b_se @ w_ih.T + b_ih + b_hh                       # [4H]
    bhp_pat = np.zeros((128, 1), np.float32)
    bhp_pat[0::32, 0] = b_hp[0]
    bhp_pat[1::32, 0] = b_hp[1]

    Wp, bp = W_eff[perm], b_eff[perm]
    Whhp, Up, b1p = w_hh[perm], U[perm], b1[perm]
    f = np.float32
    bias = np.stack([bp[0:H], bp[H:2*H], bp[2*H:3*H], bp[3*H:4*H],
                     b1p[0:H], b1p[H:2*H], b1p[2*H:3*H], b1p[3*H:4*H]],
                    axis=1)  # [H, 8]
    return {
        "wg": np.ascontiguousarray(Wp.T, f),
        "whh": np.ascontiguousarray(Whhp.T, f),
        "u": np.ascontiguousarray(Up.T, f),
        "bias": np.ascontiguousarray(bias, f),
        "whp": np.ascontiguousarray(w_hp.T, f),
        "bhp": np.ascontiguousarray(bhp_pat, f),
        "ident": np.eye(H, dtype=f),
    }


def kernel(last_pos, last_pos_rel, h0, c0,
           w_ih, w_hh, b_ih, b_hh, w_se, b_se, w_hp, b_hp):
    last_pos_rel = np.ascontiguousarray(np.asarray(last_pos_rel), np.float32)
    h0 = np.ascontiguousarray(np.asarray(h0), np.float32)
    c0 = np.ascontiguousarray(np.asarray(c0), np.float32)
    consts = _fold_weights(
        np.asarray(w_ih, np.float32), np.asarray(w_hh, np.float32),
        np.asarray(b_ih, np.float32), np.asarray(b_hh, np.float32),
        np.asarray(w_se, np.float32), np.asarray(b_se, np.float32),
        np.asarray(w_hp, np.float32), np.asarray(b_hp, np.float32),
    )

    npeds = h0.shape[0]
    npc = npeds // NCORES
    if "nc" not in _CACHE or _CACHE.get("npc") != npc:
        _CACHE["nc"] = _build_program(npc)
        _CACHE["npc"] = npc
    nc = _CACHE["nc"]

    in_maps = []
    for ci in range(NCORES):
        rows = slice(ci * npc, (ci + 1) * npc)
        m = {"h0": h0[rows], "c0": c0[rows], "lpr": last_pos_rel[rows]}
        m.update(consts)
        in_maps.append(m)

    from concourse.bass_utils import run_bass_kernel_spmd
    import os

    res = run_bass_kernel_spmd(
        nc, in_maps, list(range(NCORES)),
        tmpdir=os.environ.get("KERNEL_TRACE_DIR"),
    )
    _CACHE["exec_time_ns"] = res.exec_time_ns
    _CACHE["results"] = res
    outs = [np.asarray(res.results[i]["out"]) for i in range(NCORES)]
    return np.concatenate(outs, axis=1)
